# revision 2
# baseline (speedup 1.0000x reference)
"""Trainium2 Bass kernel for nn_HadamardProj (two-stage WHT, bf16 staging v4).

Math:
    out = -scale * (x / (||x||_2 + 1e-8)) @ proj.T + bias
    proj[o, i] = (-1)^popcount(o & i),  o < 10000, i < 2048.

proj[o, :] = H2048[o mod 2048, :]  (i < 2^11), so the projection is a
2048-point Walsh-Hadamard transform y = xn @ H2048 plus column replication
(10000 = 4*2048 + 1808) and per-row scaling r = -scale/(||x_b||+eps).

Factorization H2048 = H512 (x) H4, H512 = H4 (x) H128:  with
i = (c1*4 + c0)*128 + p and j = (jh*4 + jl)*128 + jp:

    H2048[i, j] = (-1)^pc(c1&jh) * (-1)^pc(c0&jl) * H128[p, jp]

Stage 1 (PE): per 128-row tile, 16 transposes then 16 N=512 f32r matmuls
(4 accumulation groups over c0 using the 512-wide sign-pattern LUT
lut[c0] = [s_0 H128 | .. | s_3 H128], s_q = (-1)^pc(c0&q)):

    w[:, c1*512 + jl*128 + jp] = sum_c0  xT_{c1*4+c0}.T @ lut[c0]

Stage 2 (DVE): 4-point WHT over c1, two butterfly levels; intermediates in
bf16 so level F runs in the DVE 2x packed mode.

Finals out = r*y + bias: output columns are split into three staging pieces,
one per DMA path, so the per-engine DMA chains (which serialize per issuing
engine) overlap across SP / Activation / Pool:
    piece A [0, A)        f32 staging, nc.sync   (SP HWDGE)
    piece B [A, A+B)      f32 staging, nc.scalar (ACT HWDGE)
    piece C [A+B, 10000)  bf16 staging, nc.gpsimd (Pool SWDGE, casts to f32)
A-finals: DVE fused scalar_tensor_tensor (1x, f32 out). B-finals: Pool adds
on z = r*y (z via DVE 4x tensor_scalar). C-finals: DVE fused stt in full
bf16 (2x). bias is stored broadcast in bf16 (|bias| <= 0.01, so the bf16
rounding is ~4e-5 absolute - far inside the 2e-2 gate).

Walrus limitation: a float32/float32r Matmult self-loads its weights and the
lowered S3_LW accepts a single sync-wait command.  A per-tile PE `nop` "wait
shield" absorbs every cross-engine dependency (explicit add_dep_helper edges)
so each matmul/transpose carries at most one wait.

Sharding: data-parallel, 2048 batch rows per core across 8 cores. proj is
never read (regenerated as the sign-pattern LUT host-side).
"""

import os
import sys

sys.path.insert(0, "/opt/trn_rl_repo")

import numpy as np

B_FULL = 16384
IN = 2048
OUT = 10000
N_CORES = 8
P = 128
B_CORE = B_FULL // N_CORES          # 2048 rows per core
C = IN // P                         # 16 contraction chunks
EPS = 1e-8

MM_F32R = os.environ.get("HADAMARD_MM_F32R", "1") == "1"

# Output-column pieces (cols, dma_engine, finals_engine), tuned against the
# TRN2 cost-model simulator:
#   dma_engine: sync (SP HWDGE) | scalar (ACT HWDGE) | gpsimd (Pool SWDGE)
#   finals: dve (fused scalar_tensor_tensor) | pool (adds on z)
# The gpsimd-DMA piece is staged in bf16 (SWDGE casts to f32 in flight) so
# its DVE finals run in the 2x packed mode.  Cols sum to 10000.
PIECES = [
    (3648, "sync", "pool"),
    (1600, "scalar", "pool"),
    (1152, "scalar", "dve"),
    (3600, "gpsimd", "dve"),
]
assert sum(p[0] for p in PIECES) == OUT, PIECES
IN_ENG = "sync"
WC_ENG = "vector"
ST_BUFS = 2

_CACHE = {}


def _popcount_parity(a):
    pc = np.zeros_like(a)
    n = int(a.max()).bit_length() if a.size else 1
    for k in range(max(n, 1)):
        pc += (a >> k) & 1
    return pc & 1


def _hadamard(n):
    i = np.arange(n, dtype=np.int64)
    return (1.0 - 2.0 * _popcount_parity(i[:, None] & i[None, :])).astype(np.float32)


def make_lut():
    H128 = _hadamard(P)
    lut = np.empty((4, P, 512), dtype=np.float32)
    for t in range(4):
        blocks = []
        for q in range(4):
            s = 1.0 - 2.0 * (bin(t & q).count("1") & 1)
            blocks.append(s * H128)
        lut[t] = np.concatenate(blocks, axis=1)
    return lut


def _segments(lo, hi):
    """Split out-column range [lo, hi) at 2048-block boundaries.

    Yields (out_col, y_col, width) with y_col = out_col mod 2048.
    """
    j = lo
    while j < hi:
        blk_end = (j // 2048 + 1) * 2048
        w = min(hi, blk_end) - j
        yield j, j % 2048, w
        j += w


def build_module(nb, passes=1):
    """Build the per-core Bass module processing nb 128-row tiles."""
    import concourse.bass as bass
    from concourse import bacc
    import concourse.mybir as mybir
    import concourse.tile as tile
    from concourse.tile_rust import add_dep_helper

    f32 = mybir.dt.float32
    bf16 = mybir.dt.bfloat16
    AF = mybir.ActivationFunctionType
    ALU = mybir.AluOpType

    nc = bacc.Bacc("TRN2", target_bir_lowering=False, debug=False)
    mmdt = mybir.dt.float32r if MM_F32R else f32
    x_d = nc.dram_tensor("x", [nb * P, IN], f32, kind="ExternalInput")
    lut_d = nc.dram_tensor("lut", [4, P, 512], mmdt, kind="ExternalInput")
    ident_d = nc.dram_tensor("ident", [P, P], f32, kind="ExternalInput")
    bias_d = nc.dram_tensor("biasr", [P, OUT], bf16, kind="ExternalInput")
    out_d = nc.dram_tensor("out", [nb * P, OUT], f32, kind="ExternalOutput")

    with tile.TileContext(nc) as tc:
        with (
            tc.tile_pool(name="const", bufs=1) as cp,
            tc.tile_pool(name="xin", bufs=3) as xp,
            tc.tile_pool(name="sq", bufs=1) as sqp,
            tc.tile_pool(name="xt", bufs=2) as xtp,
            tc.tile_pool(name="nrm", bufs=8) as nrmp,
            tc.tile_pool(name="ee", bufs=1) as eep,
            tc.tile_pool(name="yy", bufs=2) as yyp,
            tc.tile_pool(name="stage", bufs=ST_BUFS) as stp,
            tc.tile_pool(name="tp", bufs=2, space="PSUM") as tpp,
            tc.tile_pool(name="wp", bufs=1, space="PSUM") as wpp,
        ):
            ident = cp.tile([P, P], f32, tag="ident")
            i_dma = nc.sync.dma_start(ident[:], ident_d[:, :])
            lut = cp.tile([P, 4, 512], mmdt, tag="lut")
            l_dma = nc.sync.dma_start(lut[:], lut_d[:, :, :].rearrange("g p n -> p g n"))
            biasr = cp.tile([P, OUT], bf16, tag="biasr")
            b_dma = nc.scalar.dma_start(biasr[:], bias_d[:, :])

            prev_cross = [i_dma, l_dma, b_dma]  # deps for the next shield
            prev_wdrain = []

            for bt in [t for _ in range(passes) for t in range(nb)]:
                rows = slice(bt * P, (bt + 1) * P)

                x_t = xp.tile([P, IN], f32, tag="xtile")
                in_dma = getattr(nc, IN_ENG).dma_start(x_t[:], x_d[rows, :])

                # PE wait shield: absorbs all cross-engine waits so the f32
                # matmuls/transposes below each carry <=1 sync wait.
                shield = nc.tensor.nop(nofuse=True, hint=f"shield{bt}")
                for d in [in_dma] + prev_cross + prev_wdrain:
                    add_dep_helper(shield.ins, d.ins, reason="f32-mm wait shield")

                # r = 1 / ||x_b|| per batch row.  The -scale factor is folded
                # into the LUT (entries +-scale) and the reference's +1e-8 on
                # the norm (~45) is 2e-10 relative - dropped.
                sq = sqp.tile([P, IN], f32, tag="sq")
                s = nrmp.tile([P, 1], f32, tag="s")
                nc.scalar.activation(sq[:], x_t[:], AF.Square, accum_out=s[:])
                t = nrmp.tile([P, 1], f32, tag="t")
                nc.scalar.activation(t[:], s[:], AF.Sqrt)
                r = nrmp.tile([P, 1], f32, tag="r")
                nc.vector.reciprocal(r[:], t[:])

                # xT[p, c, b] = x[b, c*128 + p] via PE transposes (4 per group)
                heads = []
                copies = []
                xT = xtp.tile([P, C, P], mmdt, tag="xT")
                for q4 in range(4):
                    pt = tpp.tile([P, 4, P], f32, tag="pt")
                    for j in range(4):
                        c = q4 * 4 + j
                        tr = nc.tensor.matmul(
                            pt[:, j, :],
                            x_t[:, c * P : (c + 1) * P],
                            ident[:],
                            is_transpose=True,
                            start=(j == 0),
                            stop=(j == 3),
                        )
                        if j == 0:
                            heads.append(tr)
                    copies.append(nc.scalar.copy(xT[:, q4 * 4 : (q4 + 1) * 4, :], pt[:]))

                # Stage 1: w[:, c1*512 + jl*128 + jp] = sum_c0 xT_{4c1+c0}.T @ lut[c0]
                w = wpp.tile([P, 2048], f32, tag="w")
                for c1 in range(4):
                    for c0 in range(4):
                        mm = nc.tensor.matmul(
                            w[:, c1 * 512 : (c1 + 1) * 512],
                            xT[:, c1 * 4 + c0, :],
                            lut[:, c0, :],
                            start=(c0 == 0),
                            stop=(c0 == 3),
                        )
                        if c0 == 0:
                            heads.append(mm)

                for h in heads:
                    add_dep_helper(h.ins, shield.ins, reason="order after shield")

                # Stage 2: 4-point WHT over c1 (2 butterfly levels on DVE).
                # DVE may read only ONE operand from PSUM: stage the upper half
                # of w into SBUF first (DVE tensor_copy runs PSUM src at 2x).
                wc = eep.tile([P, 1024], f32, tag="wc")
                if WC_ENG == "scalar":
                    wcopy = nc.scalar.copy(wc[:], w[:, 1024:2048])
                else:
                    wcopy = nc.vector.tensor_copy(wc[:], w[:, 1024:2048])
                # level E (c1 bit1); e in bf16 so level F runs at DVE 2x.
                e = eep.tile([P, 2048], bf16, tag="e")
                e0 = nc.vector.tensor_add(e[:, 0:1024], w[:, 0:1024], wc[:])
                e1 = nc.vector.tensor_sub(e[:, 1024:2048], w[:, 0:1024], wc[:])
                # level F (c1 bit0), bf16 2x
                y = yyp.tile([P, 2048], bf16, tag="y")
                nc.vector.tensor_add(y[:, 0:512], e[:, 0:512], e[:, 512:1024])
                nc.vector.tensor_sub(y[:, 512:1024], e[:, 0:512], e[:, 512:1024])
                nc.vector.tensor_add(y[:, 1024:1536], e[:, 1024:1536], e[:, 1536:2048])
                nc.vector.tensor_sub(y[:, 1536:2048], e[:, 1024:1536], e[:, 1536:2048])

                # z = r*y (bf16, DVE 4x) for the Pool adds
                z = yyp.tile([P, 2048], bf16, tag="z")
                nc.vector.tensor_scalar_mul(z[:], y[:], r[:])

                # Finals into one staging piece per DMA path
                lo = 0
                for pi, (cols, deng, feng) in enumerate(PIECES):
                    sdt = bf16 if deng == "gpsimd" else f32
                    st = stp.tile([P, cols], sdt, tag=f"st{pi}")
                    for oc, yc, wdt in _segments(lo, lo + cols):
                        if feng == "pool":
                            nc.gpsimd.tensor_add(
                                st[:, oc - lo : oc - lo + wdt],
                                z[:, yc : yc + wdt],
                                biasr[:, oc : oc + wdt],
                            )
                        else:
                            nc.vector.scalar_tensor_tensor(
                                out=st[:, oc - lo : oc - lo + wdt],
                                in0=y[:, yc : yc + wdt],
                                scalar=r[:],
                                in1=biasr[:, oc : oc + wdt],
                                op0=ALU.mult,
                                op1=ALU.add,
                            )
                    getattr(nc, deng).dma_start(out_d[rows, lo : lo + cols], st[:])
                    lo += cols

                prev_cross = copies
                prev_wdrain = [wcopy, e0, e1]

    nc.compile()
    return nc


def get_module(nb=B_CORE // P, passes=1):
    key = ("mod", nb, MM_F32R, passes)
    if key not in _CACHE:
        _CACHE[key] = build_module(nb, passes)
    return _CACHE[key]


def make_inputs(x, scale_val, bias):
    import ml_dtypes

    lut = make_lut() * np.float32(-scale_val)   # fold -scale into the LUT
    biasr = np.ascontiguousarray(
        np.broadcast_to(bias.astype(ml_dtypes.bfloat16)[None, :], (P, OUT))
    )
    ident = np.eye(P, dtype=np.float32)
    return [
        {
            "x": x[c * B_CORE : (c + 1) * B_CORE],
            "lut": lut,
            "ident": ident,
            "biasr": biasr,
        }
        for c in range(N_CORES)
    ]


def kernel(x, proj, scale, bias):
    from concourse.bass_utils import run_bass_kernel_spmd

    x = np.ascontiguousarray(np.asarray(x, dtype=np.float32))
    bias = np.asarray(bias, dtype=np.float32)
    scale_val = float(np.asarray(scale).reshape(-1)[0])
    del proj  # deterministic +-1 Hadamard; regenerated as the sign-pattern LUT

    nc = get_module()
    in_maps = make_inputs(x, scale_val, bias)
    res = run_bass_kernel_spmd(nc, in_maps, core_ids=list(range(N_CORES)))
    return np.concatenate([res.results[c]["out"] for c in range(N_CORES)], axis=0)


# revision 3
# speedup vs baseline: 1.0327x; 1.0327x over previous
"""Trainium2 Bass kernel for nn_HadamardProj (two-stage WHT, bf16 staging v4).

Math:
    out = -scale * (x / (||x||_2 + 1e-8)) @ proj.T + bias
    proj[o, i] = (-1)^popcount(o & i),  o < 10000, i < 2048.

proj[o, :] = H2048[o mod 2048, :]  (i < 2^11), so the projection is a
2048-point Walsh-Hadamard transform y = xn @ H2048 plus column replication
(10000 = 4*2048 + 1808) and per-row scaling r = -scale/(||x_b||+eps).

Factorization H2048 = H512 (x) H4, H512 = H4 (x) H128:  with
i = (c1*4 + c0)*128 + p and j = (jh*4 + jl)*128 + jp:

    H2048[i, j] = (-1)^pc(c1&jh) * (-1)^pc(c0&jl) * H128[p, jp]

Stage 1 (PE): per 128-row tile, 16 transposes then 16 N=512 f32r matmuls
(4 accumulation groups over c0 using the 512-wide sign-pattern LUT
lut[c0] = [s_0 H128 | .. | s_3 H128], s_q = (-1)^pc(c0&q)):

    w[:, c1*512 + jl*128 + jp] = sum_c0  xT_{c1*4+c0}.T @ lut[c0]

Stage 2 (DVE): 4-point WHT over c1, two butterfly levels; intermediates in
bf16 so level F runs in the DVE 2x packed mode.

Finals out = r*y + bias: output columns are split into three staging pieces,
one per DMA path, so the per-engine DMA chains (which serialize per issuing
engine) overlap across SP / Activation / Pool:
    piece A [0, A)        f32 staging, nc.sync   (SP HWDGE)
    piece B [A, A+B)      f32 staging, nc.scalar (ACT HWDGE)
    piece C [A+B, 10000)  bf16 staging, nc.gpsimd (Pool SWDGE, casts to f32)
A-finals: DVE fused scalar_tensor_tensor (1x, f32 out). B-finals: Pool adds
on z = r*y (z via DVE 4x tensor_scalar). C-finals: DVE fused stt in full
bf16 (2x). bias is stored broadcast in bf16 (|bias| <= 0.01, so the bf16
rounding is ~4e-5 absolute - far inside the 2e-2 gate).

Walrus limitation: a float32/float32r Matmult self-loads its weights and the
lowered S3_LW accepts a single sync-wait command.  A per-tile PE `nop` "wait
shield" absorbs every cross-engine dependency (explicit add_dep_helper edges)
so each matmul/transpose carries at most one wait.

Sharding: data-parallel, 2048 batch rows per core across 8 cores. proj is
never read (regenerated as the sign-pattern LUT host-side).
"""

import os
import sys

sys.path.insert(0, "/opt/trn_rl_repo")

import numpy as np

B_FULL = 16384
IN = 2048
OUT = 10000
N_CORES = 8
P = 128
B_CORE = B_FULL // N_CORES          # 2048 rows per core
C = IN // P                         # 16 contraction chunks
EPS = 1e-8

MM_F32R = os.environ.get("HADAMARD_MM_F32R", "1") == "1"

# Output-column pieces: "cols:dma_engine:finals" comma-separated.
#   dma_engine: sync (SP HWDGE) | scalar (ACT HWDGE) | gpsimd (Pool SWDGE)
#   finals: dve (fused stt) | pool (adds on z)
# A gpsimd-DMA piece is staged in bf16 (SWDGE casts to f32) so its DVE
# finals run in the 2x packed mode.  Cols must sum to 10000.
PIECES = [
    (4224, "sync", "pool"),
    (2176, "scalar", "dve"),
    (3600, "gpsimd", "dve"),
]
assert sum(p[0] for p in PIECES) == OUT, PIECES
IN_ENG = "sync"
WC_ENG = "vector"
ST_BUFS = 2
XIN_BUFS = 3
XT_BUFS = 2
YY_BUFS = 2
EE_BUFS = 1

_CACHE = {}


def _popcount_parity(a):
    pc = np.zeros_like(a)
    n = int(a.max()).bit_length() if a.size else 1
    for k in range(max(n, 1)):
        pc += (a >> k) & 1
    return pc & 1


def _hadamard(n):
    i = np.arange(n, dtype=np.int64)
    return (1.0 - 2.0 * _popcount_parity(i[:, None] & i[None, :])).astype(np.float32)


def make_lut():
    H128 = _hadamard(P)
    lut = np.empty((4, P, 512), dtype=np.float32)
    for t in range(4):
        blocks = []
        for q in range(4):
            s = 1.0 - 2.0 * (bin(t & q).count("1") & 1)
            blocks.append(s * H128)
        lut[t] = np.concatenate(blocks, axis=1)
    return lut


def _segments(lo, hi):
    """Split out-column range [lo, hi) at 2048-block boundaries.

    Yields (out_col, y_col, width) with y_col = out_col mod 2048.
    """
    j = lo
    while j < hi:
        blk_end = (j // 2048 + 1) * 2048
        w = min(hi, blk_end) - j
        yield j, j % 2048, w
        j += w


def build_module(nb, passes=1):
    """Build the per-core Bass module processing nb 128-row tiles."""
    import concourse.bass as bass
    from concourse import bacc
    import concourse.mybir as mybir
    import concourse.tile as tile
    from concourse.tile_rust import add_dep_helper

    f32 = mybir.dt.float32
    bf16 = mybir.dt.bfloat16
    AF = mybir.ActivationFunctionType
    ALU = mybir.AluOpType

    nc = bacc.Bacc("TRN2", target_bir_lowering=False, debug=False)
    mmdt = mybir.dt.float32r if MM_F32R else f32
    x_d = nc.dram_tensor("x", [nb * P, IN], f32, kind="ExternalInput")
    lut_d = nc.dram_tensor("lut", [4, P, 512], mmdt, kind="ExternalInput")
    ident_d = nc.dram_tensor("ident", [P, P], f32, kind="ExternalInput")
    bias_d = nc.dram_tensor("biasr", [P, OUT], bf16, kind="ExternalInput")
    out_d = nc.dram_tensor("out", [nb * P, OUT], f32, kind="ExternalOutput")

    with tile.TileContext(nc) as tc:
        with (
            tc.tile_pool(name="const", bufs=1) as cp,
            tc.tile_pool(name="xin", bufs=XIN_BUFS) as xp,

            tc.tile_pool(name="xt", bufs=XT_BUFS) as xtp,
            tc.tile_pool(name="nrm", bufs=8) as nrmp,
            tc.tile_pool(name="ee", bufs=EE_BUFS) as eep,
            tc.tile_pool(name="yy", bufs=YY_BUFS) as yyp,
            tc.tile_pool(name="stage", bufs=ST_BUFS) as stp,
            tc.tile_pool(name="sq", bufs=1) as sqp,
            tc.tile_pool(name="wp", bufs=2, space="PSUM") as wpp,
        ):
            ident = cp.tile([P, P], f32, tag="ident")
            i_dma = nc.sync.dma_start(ident[:], ident_d[:, :])
            lut = cp.tile([P, 4, 512], mmdt, tag="lut")
            l_dma = nc.gpsimd.dma_start(lut[:], lut_d[:, :, :].rearrange("g p n -> p g n"))
            biasr = cp.tile([P, OUT], bf16, tag="biasr")
            b_dma = nc.scalar.dma_start(biasr[:], bias_d[:, :])

            prev_cross = [i_dma, l_dma, b_dma]  # deps for the next shield
            prev_wdrain = []
            prev_wdrain2 = []

            for bt in [t for _ in range(passes) for t in range(nb)]:
                rows = slice(bt * P, (bt + 1) * P)

                x_t = xp.tile([P, IN], f32, tag="xtile")
                ie = IN_ENG
                if ie == "rr":
                    ie = ("sync", "scalar", "gpsimd")[bt % 3]
                in_dma = getattr(nc, ie).dma_start(x_t[:], x_d[rows, :])

                # PE wait shield: absorbs all cross-engine waits so the f32
                # matmuls/transposes below each carry <=1 sync wait.
                shield = nc.tensor.nop(nofuse=True, hint=f"shield{bt}")
                for d in [in_dma] + prev_cross + prev_wdrain2:
                    add_dep_helper(shield.ins, d.ins, reason="f32-mm wait shield")

                # r = 1 / ||x_b|| per batch row (-scale lives in the LUT;
                # the reference's +1e-8 on the ~45 norm is 2e-10 - dropped).
                sq = sqp.tile([P, IN], f32, tag="sq")
                s = nrmp.tile([P, 1], f32, tag="s")
                nc.scalar.activation(sq[:], x_t[:], AF.Square, accum_out=s[:])
                t = nrmp.tile([P, 1], f32, tag="t")
                nc.scalar.activation(t[:], s[:], AF.Sqrt)
                r = nrmp.tile([P, 1], f32, tag="r")
                nc.vector.reciprocal(r[:], t[:])

                # xT[p, c, b] = x[b, c*128 + p] via PE transposes (4 per group)
                heads = []
                copies = []
                xT = xtp.tile([P, C, P], mmdt, tag="xT")
                w = wpp.tile([P, 2048], f32, tag="w")
                wv = w.rearrange("p (c n) -> p c n", c=C)
                for q4 in range(4):
                    for j in range(4):
                        c = q4 * 4 + j
                        tr = nc.tensor.matmul(
                            wv[:, c, :],
                            x_t[:, c * P : (c + 1) * P],
                            ident[:],
                            is_transpose=True,
                            start=(j == 0),
                            stop=(j == 3),
                        )
                        if j == 0:
                            heads.append(tr)
                    copies.append(
                        nc.scalar.copy(
                            xT[:, q4 * 4 : (q4 + 1) * 4, :],
                            wv[:, q4 * 4 : (q4 + 1) * 4, :],
                        )
                    )

                # Stage 1: w[:, c1*512 + jl*128 + jp] = sum_c0 xT_{4c1+c0}.T @ lut[c0]
                # (overwrites the transpose staging banks, in accumulation
                # groups, after each bank's copy has drained it)
                for c1 in range(4):
                    for c0 in range(4):
                        mm = nc.tensor.matmul(
                            w[:, c1 * 512 : (c1 + 1) * 512],
                            xT[:, c1 * 4 + c0, :],
                            lut[:, c0, :],
                            start=(c0 == 0),
                            stop=(c0 == 3),
                        )
                        if c0 == 0:
                            heads.append(mm)

                for h in heads:
                    add_dep_helper(h.ins, shield.ins, reason="order after shield")

                # Stage 2: 4-point WHT over c1 (2 butterfly levels on DVE).
                # DVE may read only ONE operand from PSUM: stage the upper half
                # of w into SBUF first (DVE tensor_copy runs PSUM src at 2x).
                # Fold r into level E (linearity: r*F(E(w)) = F(E(r*w))):
                # wc' = r * w_hi (PSUM->SBUF bf16, same cost as the plain
                # copy) and E runs as scalar_tensor_tensor with scalar=r, so
                # y comes out pre-scaled and no separate z op is needed.
                wc = eep.tile([P, 1024], bf16, tag="wc")
                wcopy = nc.vector.tensor_scalar_mul(wc[:], w[:, 1024:2048], r[:])
                # level E (c1 bit1); e in bf16 so level F runs at DVE 2x.
                e = eep.tile([P, 2048], bf16, tag="e")
                e0 = nc.vector.scalar_tensor_tensor(
                    out=e[:, 0:1024], in0=w[:, 0:1024], scalar=r[:], in1=wc[:],
                    op0=ALU.mult, op1=ALU.add)
                e1 = nc.vector.scalar_tensor_tensor(
                    out=e[:, 1024:2048], in0=w[:, 0:1024], scalar=r[:], in1=wc[:],
                    op0=ALU.mult, op1=ALU.subtract)
                # level F (c1 bit0), bf16 2x, merged into 2 strided-AP ops
                y = yyp.tile([P, 2, 2, 512], bf16, tag="y")
                e4 = e.rearrange("p (a b n) -> p a b n", a=2, b=2)
                nc.vector.tensor_add(y[:, :, 0, :], e4[:, :, 0, :], e4[:, :, 1, :])
                nc.vector.tensor_sub(y[:, :, 1, :], e4[:, :, 0, :], e4[:, :, 1, :])
                y = y.rearrange("p a b n -> p (a b n)")

                # Finals into one staging piece per DMA path
                lo = 0
                for pi, (cols, deng, feng) in enumerate(PIECES):
                    sdt = bf16 if deng == "gpsimd" else f32
                    st = stp.tile([P, cols], sdt, tag=f"st{pi}")
                    for oc, yc, wdt in _segments(lo, lo + cols):
                        eng = nc.gpsimd if feng == "pool" else nc.vector
                        eng.tensor_add(
                            st[:, oc - lo : oc - lo + wdt],
                            y[:, yc : yc + wdt],
                            biasr[:, oc : oc + wdt],
                        )
                    getattr(nc, deng).dma_start(out_d[rows, lo : lo + cols], st[:])
                    lo += cols

                prev_cross = copies
                prev_wdrain2 = prev_wdrain
                prev_wdrain = [wcopy, e0, e1]

    nc.compile()
    return nc


def get_module(nb=B_CORE // P, passes=1):
    key = ("mod", nb, MM_F32R, passes)
    if key not in _CACHE:
        _CACHE[key] = build_module(nb, passes)
    return _CACHE[key]


def make_inputs(x, scale_val, bias):
    import ml_dtypes

    lut = make_lut() * np.float32(-scale_val)   # fold -scale into the LUT
    biasr = np.ascontiguousarray(
        np.broadcast_to(bias.astype(ml_dtypes.bfloat16)[None, :], (P, OUT))
    )
    ident = np.eye(P, dtype=np.float32)
    return [
        {
            "x": x[c * B_CORE : (c + 1) * B_CORE],
            "lut": lut,
            "ident": ident,
            "biasr": biasr,
        }
        for c in range(N_CORES)
    ]


def kernel(x, proj, scale, bias):
    from concourse.bass_utils import run_bass_kernel_spmd

    x = np.ascontiguousarray(np.asarray(x, dtype=np.float32))
    bias = np.asarray(bias, dtype=np.float32)
    scale_val = float(np.asarray(scale).reshape(-1)[0])
    del proj  # deterministic +-1 Hadamard; regenerated as the sign-pattern LUT

    nc = get_module()
    in_maps = make_inputs(x, scale_val, bias)
    res = run_bass_kernel_spmd(nc, in_maps, core_ids=list(range(N_CORES)))
    return np.concatenate([res.results[c]["out"] for c in range(N_CORES)], axis=0)


# revision 4
# speedup vs baseline: 1.0761x; 1.0420x over previous
"""Trainium2 Bass kernel for nn_HadamardProj (two-stage WHT, bf16 staging v4).

Math:
    out = -scale * (x / (||x||_2 + 1e-8)) @ proj.T + bias
    proj[o, i] = (-1)^popcount(o & i),  o < 10000, i < 2048.

proj[o, :] = H2048[o mod 2048, :]  (i < 2^11), so the projection is a
2048-point Walsh-Hadamard transform y = xn @ H2048 plus column replication
(10000 = 4*2048 + 1808) and per-row scaling r = -scale/(||x_b||+eps).

Factorization H2048 = H512 (x) H4, H512 = H4 (x) H128:  with
i = (c1*4 + c0)*128 + p and j = (jh*4 + jl)*128 + jp:

    H2048[i, j] = (-1)^pc(c1&jh) * (-1)^pc(c0&jl) * H128[p, jp]

Stage 1 (PE): per 128-row tile, 16 transposes then 16 N=512 f32r matmuls
(4 accumulation groups over c0 using the 512-wide sign-pattern LUT
lut[c0] = [s_0 H128 | .. | s_3 H128], s_q = (-1)^pc(c0&q)):

    w[:, c1*512 + jl*128 + jp] = sum_c0  xT_{c1*4+c0}.T @ lut[c0]

Stage 2 (DVE): 4-point WHT over c1, two butterfly levels; intermediates in
bf16 so level F runs in the DVE 2x packed mode.

Finals out = r*y + bias: output columns are split into three staging pieces,
one per DMA path, so the per-engine DMA chains (which serialize per issuing
engine) overlap across SP / Activation / Pool:
    piece A [0, A)        f32 staging, nc.sync   (SP HWDGE)
    piece B [A, A+B)      f32 staging, nc.scalar (ACT HWDGE)
    piece C [A+B, 10000)  bf16 staging, nc.gpsimd (Pool SWDGE, casts to f32)
A-finals: DVE fused scalar_tensor_tensor (1x, f32 out). B-finals: Pool adds
on z = r*y (z via DVE 4x tensor_scalar). C-finals: DVE fused stt in full
bf16 (2x). bias is stored broadcast in bf16 (|bias| <= 0.01, so the bf16
rounding is ~4e-5 absolute - far inside the 2e-2 gate).

Walrus limitation: a float32/float32r Matmult self-loads its weights and the
lowered S3_LW accepts a single sync-wait command.  A per-tile PE `nop` "wait
shield" absorbs every cross-engine dependency (explicit add_dep_helper edges)
so each matmul/transpose carries at most one wait.

Sharding: data-parallel, 2048 batch rows per core across 8 cores. proj is
never read (regenerated as the sign-pattern LUT host-side).
"""

import os
import sys

sys.path.insert(0, "/opt/trn_rl_repo")

import numpy as np

B_FULL = 16384
IN = 2048
OUT = 10000
N_CORES = 8
P = 128
B_CORE = B_FULL // N_CORES          # 2048 rows per core
C = IN // P                         # 16 contraction chunks
EPS = 1e-8

MM_F32R = os.environ.get("HADAMARD_MM_F32R", "1") == "1"

# Output-column pieces: "cols:dma_engine:finals" comma-separated.
#   dma_engine: sync (SP HWDGE) | scalar (ACT HWDGE) | gpsimd (Pool SWDGE)
#   finals: dve (fused stt) | pool (adds on z)
# A gpsimd-DMA piece is staged in bf16 (SWDGE casts to f32) so its DVE
# finals run in the 2x packed mode.  Cols must sum to 10000.
PIECES = [
    (4224, "sync", "pool"),
    (2176, "scalar", "dve"),
    (3600, "gpsimd", "dve"),
]
assert sum(p[0] for p in PIECES) == OUT, PIECES
IN_ENG = "sync"
WC_ENG = "vector"
ST_BUFS = 2
XIN_BUFS = 3
XT_BUFS = 2
YY_BUFS = 2
EE_BUFS = 1

_CACHE = {}


def _popcount_parity(a):
    pc = np.zeros_like(a)
    n = int(a.max()).bit_length() if a.size else 1
    for k in range(max(n, 1)):
        pc += (a >> k) & 1
    return pc & 1


def _hadamard(n):
    i = np.arange(n, dtype=np.int64)
    return (1.0 - 2.0 * _popcount_parity(i[:, None] & i[None, :])).astype(np.float32)


def make_lut():
    H128 = _hadamard(P)
    lut = np.empty((4, P, 512), dtype=np.float32)
    for t in range(4):
        blocks = []
        for q in range(4):
            s = 1.0 - 2.0 * (bin(t & q).count("1") & 1)
            blocks.append(s * H128)
        lut[t] = np.concatenate(blocks, axis=1)
    return lut


def _segments(lo, hi):
    """Split out-column range [lo, hi) at 2048-block boundaries.

    Yields (out_col, y_col, width) with y_col = out_col mod 2048.
    """
    j = lo
    while j < hi:
        blk_end = (j // 2048 + 1) * 2048
        w = min(hi, blk_end) - j
        yield j, j % 2048, w
        j += w


def build_module(nb, passes=1):
    """Build the per-core Bass module processing nb 128-row tiles."""
    import concourse.bass as bass
    from concourse import bacc
    import concourse.mybir as mybir
    import concourse.tile as tile
    from concourse.tile_rust import add_dep_helper

    f32 = mybir.dt.float32
    bf16 = mybir.dt.bfloat16
    AF = mybir.ActivationFunctionType
    ALU = mybir.AluOpType

    nc = bacc.Bacc("TRN2", target_bir_lowering=False, debug=False)
    mmdt = mybir.dt.float32r if MM_F32R else f32
    x_d = nc.dram_tensor("x", [nb * P, IN], f32, kind="ExternalInput")
    lut_d = nc.dram_tensor("lut", [4, P, 512], mmdt, kind="ExternalInput")
    ident_d = nc.dram_tensor("ident", [P, P], f32, kind="ExternalInput")
    bias_d = nc.dram_tensor("biasr", [P, OUT], bf16, kind="ExternalInput")
    out_d = nc.dram_tensor("out", [nb * P, OUT], f32, kind="ExternalOutput")

    with tile.TileContext(nc) as tc:
        with (
            tc.tile_pool(name="const", bufs=1) as cp,
            tc.tile_pool(name="xin", bufs=XIN_BUFS) as xp,

            tc.tile_pool(name="xt", bufs=XT_BUFS) as xtp,
            tc.tile_pool(name="nrm", bufs=8) as nrmp,
            tc.tile_pool(name="ee", bufs=EE_BUFS) as eep,
            tc.tile_pool(name="yy", bufs=YY_BUFS) as yyp,
            tc.tile_pool(name="stage", bufs=ST_BUFS) as stp,
            tc.tile_pool(name="sq", bufs=1) as sqp,
            tc.tile_pool(name="wp", bufs=2, space="PSUM") as wpp,
        ):
            ident = cp.tile([P, P], f32, tag="ident")
            i_dma = nc.sync.dma_start(ident[:], ident_d[:, :])
            lut = cp.tile([P, 4, 512], mmdt, tag="lut")
            l_dma = nc.gpsimd.dma_start(lut[:], lut_d[:, :, :].rearrange("g p n -> p g n"))
            biasr = cp.tile([P, OUT], bf16, tag="biasr")
            b_dma = nc.scalar.dma_start(biasr[:], bias_d[:, :])

            prev_cross = [i_dma, l_dma, b_dma]  # deps for the next shield
            prev_cross2 = []
            prev_wdrain = []
            prev_wdrain2 = []

            for bt in [t for _ in range(passes) for t in range(nb)]:
                rows = slice(bt * P, (bt + 1) * P)

                x_tt = xp.tile([P, IN], f32, tag="xtile")
                in_dma = nc.sync.dma_start(x_tt[:], x_d[rows, :])
                x_t = x_tt[:]

                # PE wait shield: absorbs all cross-engine waits so the f32
                # matmuls/transposes below each carry <=1 sync wait.
                shield = nc.tensor.nop(nofuse=True, hint=f"shield{bt}")
                # 2-back deps: tile t's transposes overwrite the w-buffer
                # whose last readers are the copies and wc/e ops of tile t-2
                # (w is double-buffered; the old 1-back copies dep was a
                # leftover from the separate pt staging).
                for d in [in_dma] + prev_cross2 + prev_wdrain2:
                    add_dep_helper(shield.ins, d.ins, reason="f32-mm wait shield")

                # r = 1 / ||x_b|| per batch row (-scale lives in the LUT;
                # the reference's +1e-8 on the ~45 norm is 2e-10 - dropped).
                sq = sqp.tile([P, IN], f32, tag="sq")
                s = nrmp.tile([P, 1], f32, tag="s")
                nc.scalar.activation(sq[:], x_t, AF.Square, accum_out=s[:])
                t = nrmp.tile([P, 1], f32, tag="t")
                nc.scalar.activation(t[:], s[:], AF.Sqrt)
                r = nrmp.tile([P, 1], f32, tag="r")
                nc.vector.reciprocal(r[:], t[:])

                # xT[p, c, b] = x[b, c*128 + p] via PE transposes (4 per group)
                heads = []
                copies = []
                xT = xtp.tile([P, C, P], mmdt, tag="xT")
                w = wpp.tile([P, 2048], f32, tag="w")
                wv = w.rearrange("p (c n) -> p c n", c=C)
                for q4 in range(4):
                    for j in range(4):
                        c = q4 * 4 + j
                        tr = nc.tensor.matmul(
                            wv[:, c, :],
                            x_t[:, c * P : (c + 1) * P],
                            ident[:],
                            is_transpose=True,
                            start=(j == 0),
                            stop=(j == 3),
                        )
                        if j == 0:
                            heads.append(tr)
                    copies.append(
                        nc.scalar.copy(
                            xT[:, q4 * 4 : (q4 + 1) * 4, :],
                            wv[:, q4 * 4 : (q4 + 1) * 4, :],
                        )
                    )

                # Stage 1: w[:, c1*512 + jl*128 + jp] = sum_c0 xT_{4c1+c0}.T @ lut[c0]
                # (overwrites the transpose staging banks, in accumulation
                # groups, after each bank's copy has drained it)
                for c1 in range(4):
                    for c0 in range(4):
                        mm = nc.tensor.matmul(
                            w[:, c1 * 512 : (c1 + 1) * 512],
                            xT[:, c1 * 4 + c0, :],
                            lut[:, c0, :],
                            start=(c0 == 0),
                            stop=(c0 == 3),
                        )
                        if c0 == 0:
                            heads.append(mm)

                for h in heads:
                    add_dep_helper(h.ins, shield.ins, reason="order after shield")

                # Stage 2: 4-point WHT over c1 (2 butterfly levels on DVE).
                # DVE may read only ONE operand from PSUM: stage the upper half
                # of w into SBUF first (DVE tensor_copy runs PSUM src at 2x).
                # Fold r into level E (linearity: r*F(E(w)) = F(E(r*w))):
                # wc' = r * w_hi (PSUM->SBUF bf16, same cost as the plain
                # copy) and E runs as scalar_tensor_tensor with scalar=r, so
                # y comes out pre-scaled and no separate z op is needed.
                wc = eep.tile([P, 1024], bf16, tag="wc")
                wcopy = nc.vector.tensor_scalar_mul(wc[:], w[:, 1024:2048], r[:])
                # level E (c1 bit1); e in bf16 so level F runs at DVE 2x.
                e = eep.tile([P, 2048], bf16, tag="e")
                e0 = nc.vector.scalar_tensor_tensor(
                    out=e[:, 0:1024], in0=w[:, 0:1024], scalar=r[:], in1=wc[:],
                    op0=ALU.mult, op1=ALU.add)
                e1 = nc.vector.scalar_tensor_tensor(
                    out=e[:, 1024:2048], in0=w[:, 0:1024], scalar=r[:], in1=wc[:],
                    op0=ALU.mult, op1=ALU.subtract)
                # level F (c1 bit0), bf16 2x, merged into 2 strided-AP ops
                y = yyp.tile([P, 2, 2, 512], bf16, tag="y")
                e4 = e.rearrange("p (a b n) -> p a b n", a=2, b=2)
                nc.vector.tensor_add(y[:, :, 0, :], e4[:, :, 0, :], e4[:, :, 1, :])
                nc.vector.tensor_sub(y[:, :, 1, :], e4[:, :, 0, :], e4[:, :, 1, :])
                y = y.rearrange("p a b n -> p (a b n)")

                # Finals into one staging piece per DMA path
                lo = 0
                for pi, (cols, deng, feng) in enumerate(PIECES):
                    sdt = bf16 if deng == "gpsimd" else f32
                    st = stp.tile([P, cols], sdt, tag=f"st{pi}")
                    for oc, yc, wdt in _segments(lo, lo + cols):
                        eng = nc.gpsimd if feng == "pool" else nc.vector
                        eng.tensor_add(
                            st[:, oc - lo : oc - lo + wdt],
                            y[:, yc : yc + wdt],
                            biasr[:, oc : oc + wdt],
                        )
                    getattr(nc, deng).dma_start(out_d[rows, lo : lo + cols], st[:])
                    lo += cols

                prev_cross2 = prev_cross
                prev_cross = copies
                prev_wdrain2 = prev_wdrain
                prev_wdrain = [wcopy, e0, e1]

    nc.compile()
    return nc


def get_module(nb=B_CORE // P, passes=1):
    key = ("mod", nb, MM_F32R, passes)
    if key not in _CACHE:
        _CACHE[key] = build_module(nb, passes)
    return _CACHE[key]


def make_inputs(x, scale_val, bias):
    import ml_dtypes

    lut = make_lut() * np.float32(-scale_val)   # fold -scale into the LUT
    biasr = np.ascontiguousarray(
        np.broadcast_to(bias.astype(ml_dtypes.bfloat16)[None, :], (P, OUT))
    )
    ident = np.eye(P, dtype=np.float32)
    return [
        {
            "x": x[c * B_CORE : (c + 1) * B_CORE],
            "lut": lut,
            "ident": ident,
            "biasr": biasr,
        }
        for c in range(N_CORES)
    ]


def kernel(x, proj, scale, bias):
    from concourse.bass_utils import run_bass_kernel_spmd

    x = np.ascontiguousarray(np.asarray(x, dtype=np.float32))
    bias = np.asarray(bias, dtype=np.float32)
    scale_val = float(np.asarray(scale).reshape(-1)[0])
    del proj  # deterministic +-1 Hadamard; regenerated as the sign-pattern LUT

    nc = get_module()
    in_maps = make_inputs(x, scale_val, bias)
    res = run_bass_kernel_spmd(nc, in_maps, core_ids=list(range(N_CORES)))
    return np.concatenate([res.results[c]["out"] for c in range(N_CORES)], axis=0)
